# revision 3
# baseline (speedup 1.0000x reference)
"""Trainium2 Bass kernel for nn_KernelGraphCalcLayer (GNN message passing).

Computation (per batch b):
    h = relu(node_feats @ weight + bias)            # (N, OUT_DIM)
    h = h.reshape(N, K, DK)
    out[n, k, d] = sum_m adj[k, n, m] * h[m, k, d]  # per-kernel dense aggregation

Sharding: batch dim (64) split across 8 NeuronCores, 8 batches per core.
No cross-device communication.

Per-core dataflow (DMA-bound problem: 16MB adj + 4MB x + 1MB W reads,
2MB bf16 out writes per core):
  - adj loads split across BOTH HWDGE queues (sync + scalar), 2 k-pairs
    per queue per batch.  Rows are packed 2-per-partition (rows 2p, 2p+1
    are HBM-contiguous -> one 2KB descriptor per partition per k), so each
    pair-DMA is 256 x 2KB descriptors.  This keeps per-queue issue work
    (~0.8us/inst) far below transfer time and lets adj stream at the
    aggregate DMA rate instead of a single queue's issue-limited rate.
  - The 2-row packing makes transposed-adj free columns map to nodes
    2p+two (even/odd interleaved); aggregation psum banks then hold
    even/odd node tiles and the store uses a strided row view.
  - x (4MB) + W load via SWDGE cast-DMA to bf16 (W first: the batch-0
    linear is an early PE-program dependency); out stores (bf16, halves
    write traffic; host upcasts) also ride SWDGE so the HWDGE queues
    carry nothing but adj.
  - Per-batch PE program order: all 8 adj-transpose groups FIRST (in
    pair-arrival order), then xT + linear, then the 32 aggregation
    matmuls.  Transposes depend only on the adj DMAs, so PE never blocks
    adj-tile recycling behind W/h dependencies, and by the time the
    matmuls issue their aT drains have long completed (no PE->DVE->PE
    round-trip bubble per k).
"""

import numpy as np

import concourse.bass as bass
import concourse.mybir as mybir
from concourse import bacc
import concourse.tile as tile
from concourse.bass_utils import run_bass_kernel_spmd
from concourse.masks import make_identity

B, N, IN_DIM, OUT_DIM, K = 64, 256, 512, 512, 8
DK = OUT_DIM // K
N_CORES = 8
BPC = B // N_CORES  # batches per core

FP32 = mybir.dt.float32
FP32R = mybir.dt.float32r
BF16 = mybir.dt.bfloat16
CDT = mybir.dt.bfloat16  # compute dtype for matmul operands
P = 128  # SBUF partitions

_compiled = {}


def _build(cdt=CDT):
    nc = bacc.Bacc("TRN2", target_bir_lowering=False, debug=False)
    x_ap = nc.dram_tensor("node_feats", [BPC, N, IN_DIM], FP32, kind="ExternalInput").ap()
    adj_ap = nc.dram_tensor("adj", [BPC, K, N, N], FP32R, kind="ExternalInput").ap()
    w_ap = nc.dram_tensor("weight", [IN_DIM, OUT_DIM], FP32, kind="ExternalInput").ap()
    b_ap = nc.dram_tensor("bias", [OUT_DIM], FP32, kind="ExternalInput").ap()
    out_ap = nc.dram_tensor("out", [BPC, N, OUT_DIM], BF16, kind="ExternalOutput").ap()

    NC2 = N // P       # 2 node chunks of 128
    IC4 = IN_DIM // P  # 4 input-feature chunks
    NPAIR = K // 2     # 4 k-pairs per batch
    PF = 3             # batches of prefetch issued ahead
    # process k in pair-arrival order: sync delivers kp0 then kp1,
    # scalar kp2 then kp3, roughly interleaved in time
    KORDER = [0, 1, 4, 5, 2, 3, 6, 7]

    with tile.TileContext(nc) as tc:
        with (
            tc.tile_pool(name="singles", bufs=1) as singles,
            tc.tile_pool(name="p_x", bufs=4) as p_x,
            tc.tile_pool(name="p_xt", bufs=4) as p_xt,
            tc.tile_pool(name="p_h", bufs=4) as p_h,
            tc.tile_pool(name="p_adj", bufs=16) as p_adj,
            tc.tile_pool(name="p_adjt", bufs=12) as p_adjt,
            tc.tile_pool(name="p_out", bufs=4) as p_out,
            tc.tile_pool(name="ps_ta", bufs=4, space=bass.MemorySpace.PSUM) as ps_ta,
            tc.tile_pool(name="ps_h", bufs=2, space=bass.MemorySpace.PSUM) as ps_h,
            tc.tile_pool(name="ps_o", bufs=2, space=bass.MemorySpace.PSUM) as ps_o,
        ):
            # --- constants ---
            id_src = singles.tile([P, P], FP32)
            make_identity(nc, id_src[:])
            id_f = singles.tile([P, P], FP32R)    # identity for fp32r transposes
            nc.vector.tensor_copy(id_f[:], id_src[:])
            id_c = singles.tile([P, P], cdt)      # identity for bf16 transposes
            make_identity(nc, id_c[:])
            ones_row = singles.tile([1, P], cdt)
            nc.gpsimd.memset(ones_row[:], 1.0)
            bias_c = singles.tile([1, OUT_DIM], cdt)
            nc.gpsimd.dma_start(out=bias_c[:], in_=b_ap[None, :])
            w_sb = [singles.tile([P, OUT_DIM], cdt, name=f"w{ic}")
                    for ic in range(IC4)]
            # W before any x: the batch-0 linear blocks the in-order PE
            # program, so W must be resident early
            for ic in range(IC4):
                nc.gpsimd.dma_start(
                    out=w_sb[ic][:], in_=w_ap[ic * P:(ic + 1) * P, :])

            # DRAM views
            # x: [BPC, 128, 2, 512]; partition p <- nodes p, 128+p
            x_v = x_ap.rearrange("b (c p) i -> b p c i", p=P)
            # adj: [BPC, 128, K, 512]; partition p <- rows 2p, 2p+1 of each
            # k slice (contiguous 2KB in HBM)
            adj_v = adj_ap.rearrange("b k (p two) m -> b p k (two m)", two=2)
            # out: [BPC, 2, 128, OUT]; parity-two tile row p <- node 2p+two
            out_v = out_ap.rearrange("b (p two) o -> b two p o", two=2)

            pref = {}

            def prefetch(b):
                a_sbs = []
                for kp in range(NPAIR):
                    eng = nc.sync if kp < NPAIR // 2 else nc.scalar
                    t = p_adj.tile([P, 2 * 2 * N], FP32R, tag="adj",
                                   name=f"a{b}_{kp}")
                    eng.dma_start(out=t[:], in_=adj_v[b, :, 2 * kp:2 * kp + 2])
                    a_sbs.append(t)
                x_sb = p_x.tile([P, NC2 * IN_DIM], cdt, tag="x", name=f"x{b}")
                nc.gpsimd.dma_start(out=x_sb[:], in_=x_v[b])
                pref[b] = (a_sbs, x_sb)

            for b in range(PF):
                prefetch(b)

            cast_rr = 0  # round-robin DVE/ACT for adjT casts

            for b in range(BPC):
                if b + PF < BPC:
                    prefetch(b + PF)
                a_sbs, x_sb = pref.pop(b)

                # --- adj transposes first: consume adj tiles in arrival
                # order, independent of W/x/h ---
                aTs = {}
                for k in KORDER:
                    kp, kl = divmod(k, 2)
                    a_sb = a_sbs[kp]
                    aT = p_adjt.tile([P, 4 * P], cdt, tag="adjT",
                                     name=f"aT{b}_{k}")
                    pt = ps_ta.tile([P, 4 * P], FP32R, tag="pstf",
                                    name=f"pta{b}_{k}")
                    for two in range(2):
                        for j in range(2):
                            blk = (two * 2 + j) * P
                            src = kl * 2 * N + two * N + j * P
                            nc.tensor.transpose(
                                pt[:, blk:blk + P],
                                a_sb[:, src:src + P],
                                id_f[:])
                    if cast_rr % 4 == 1:
                        nc.scalar.copy(aT[:], pt[:])
                    else:
                        nc.vector.tensor_copy(aT[:], pt[:])
                    cast_rr += 1
                    aTs[k] = aT

                # --- transpose x -> xT packed by node-chunk (bf16) ---
                xTn = []
                for nch in range(NC2):
                    t = p_xt.tile([P, IC4 * P], cdt, tag="xT",
                                  name=f"xT{b}_{nch}")
                    pt = ps_ta.tile([P, IC4 * P], cdt, tag="pstf",
                                    name=f"ptx{b}_{nch}")
                    for ic in range(IC4):
                        nc.tensor.transpose(
                            pt[:, ic * P:(ic + 1) * P],
                            x_sb[:, nch * IN_DIM + ic * P:
                                 nch * IN_DIM + (ic + 1) * P],
                            id_c[:])
                    nc.vector.tensor_copy(t[:], pt[:])
                    xTn.append(t)

                # --- linear + bias + relu -> h bf16 [128(n), 512(o)] x2 ---
                h_sb = []
                for nch in range(NC2):
                    ph = ps_h.tile([P, OUT_DIM], FP32, tag="psh", name=f"ph{b}_{nch}")
                    nc.tensor.matmul(ph[:], ones_row[:], bias_c[:],
                                     start=True, stop=False)
                    for ic in range(IC4):
                        nc.tensor.matmul(
                            ph[:], xTn[nch][:, ic * P:(ic + 1) * P], w_sb[ic][:],
                            start=False, stop=(ic == IC4 - 1))
                    ht = p_h.tile([P, OUT_DIM], cdt, tag="h", name=f"h{b}_{nch}")
                    nc.scalar.activation(ht[:], ph[:],
                                         mybir.ActivationFunctionType.Relu)
                    h_sb.append(ht)

                # --- per-kernel aggregation (aT drains done long ago) ---
                # po[two][p, k*DK+d] = out[node 2p+two, k*DK+d]
                po = [ps_o.tile([P, OUT_DIM], FP32, tag="pso", name=f"po{b}_{i}")
                      for i in range(2)]
                for k in KORDER:
                    aT = aTs[k]
                    for two in range(2):
                        for j in range(2):
                            blk = (two * 2 + j) * P
                            nc.tensor.matmul(
                                po[two][:, k * DK:(k + 1) * DK],
                                aT[:, blk:blk + P],
                                h_sb[j][:, k * DK:(k + 1) * DK],
                                start=(j == 0), stop=(j == 1))

                # --- drain accumulators (cast bf16) + store via SWDGE ---
                for two in range(2):
                    ot = p_out.tile([P, OUT_DIM], cdt, tag="o", name=f"o{b}_{two}")
                    nc.vector.tensor_copy(ot[:], po[two][:])
                    nc.gpsimd.dma_start(out=out_v[b, two], in_=ot[:])

    nc.compile()
    return nc


def _get_nc():
    if "nc" not in _compiled:
        _compiled["nc"] = _build()
    return _compiled["nc"]


def _run(inputs, trace=False, trace_cores=None):
    nc = _get_nc()
    node_feats = np.ascontiguousarray(inputs["node_feats"], dtype=np.float32)
    adj = np.ascontiguousarray(inputs["adj"], dtype=np.float32)
    weight = np.ascontiguousarray(inputs["weight"], dtype=np.float32)
    bias = np.ascontiguousarray(inputs["bias"], dtype=np.float32)
    in_maps = []
    for c in range(N_CORES):
        sl = slice(c * BPC, (c + 1) * BPC)
        in_maps.append({
            "node_feats": node_feats[sl],
            "adj": adj[sl],
            "weight": weight,
            "bias": bias,
        })
    res = run_bass_kernel_spmd(
        nc, in_maps, core_ids=list(range(N_CORES)),
        trace=trace, trace_cores=trace_cores)
    out = np.concatenate(
        [np.asarray(res.results[c]["out"]).astype(np.float32)
         for c in range(N_CORES)], axis=0)
    return out.reshape(B, N, OUT_DIM), res


def kernel(**inputs) -> np.ndarray:
    return _run(inputs, trace=False)[0]


# revision 4
# speedup vs baseline: 1.2060x; 1.2060x over previous
"""Trainium2 Bass kernel for nn_KernelGraphCalcLayer (GNN message passing).

Computation (per batch b):
    h = relu(node_feats @ weight + bias)            # (N, OUT_DIM)
    h = h.reshape(N, K, DK)
    out[n, k, d] = sum_m adj[k, n, m] * h[m, k, d]  # per-kernel dense aggregation

Sharding: batch dim (64) split across 8 NeuronCores, 8 batches per core.
No cross-device communication.

Per-core dataflow (DMA-bound: 16MB adj + 4MB x + 1MB W reads, 2MB bf16
out writes per core):
  - adj loads split across BOTH HWDGE queues (sync + scalar), 2 k-pairs
    per queue per batch.  Rows are packed 2-per-partition (rows 2p, 2p+1
    are HBM-contiguous -> one 2KB descriptor per partition per k), so each
    pair-DMA is 256 x 2KB descriptors: per-queue issue work stays far
    below transfer time and adj streams at the aggregate DMA rate.
  - The 2-row packing makes transposed-adj free columns map to nodes
    2p+two (even/odd interleaved); aggregation psum banks hold even/odd
    node tiles and the store uses a strided row view.
  - x + W load via SWDGE cast-DMA to bf16 (W first: the batch-0 linear is
    an early in-order-PE dependency); bf16 out stores also ride SWDGE so
    the HWDGE queues carry nothing but adj.
  - Per-batch PE order: xT, linear, then adj-transpose groups software-
    pipelined 2 ahead of the aggregation matmuls (T0 T1 T2 M0 T3 M1 ...):
    each aT drain has ~1us to land before its matmuls, so no PE<->DVE
    round-trip bubble, and LDWEIGHTS bursts stay interleaved with matmul
    streams (dense transpose blocks trip the power throttle).
  - Engine split: DVE owns only xT/aT drains (nothing queued on DVE ever
    waits on aggregation matmuls -- a po-cast there stalls the next
    batch's transpose drains); ScalarE does relu, 2 aT drains, po casts;
    SWDGE stores follow the casts.
"""

import numpy as np

import concourse.bass as bass
import concourse.mybir as mybir
from concourse import bacc
import concourse.tile as tile
from concourse.bass_utils import run_bass_kernel_spmd
from concourse.masks import make_identity

B, N, IN_DIM, OUT_DIM, K = 64, 256, 512, 512, 8
DK = OUT_DIM // K
N_CORES = 8
BPC = B // N_CORES  # batches per core

FP32 = mybir.dt.float32
FP32R = mybir.dt.float32r
BF16 = mybir.dt.bfloat16
CDT = mybir.dt.bfloat16  # compute dtype for matmul operands
P = 128  # SBUF partitions

_compiled = {}


def _build(cdt=CDT):
    nc = bacc.Bacc("TRN2", target_bir_lowering=False, debug=False)
    x_ap = nc.dram_tensor("node_feats", [BPC, N, IN_DIM], FP32, kind="ExternalInput").ap()
    adj_ap = nc.dram_tensor("adj", [BPC, K, N, N], FP32R, kind="ExternalInput").ap()
    w_ap = nc.dram_tensor("weight", [IN_DIM, OUT_DIM], FP32, kind="ExternalInput").ap()
    b_ap = nc.dram_tensor("bias", [OUT_DIM], FP32, kind="ExternalInput").ap()
    out_ap = nc.dram_tensor("out", [BPC, N, OUT_DIM], BF16, kind="ExternalOutput").ap()

    NC2 = N // P       # 2 node chunks of 128
    IC4 = IN_DIM // P  # 4 input-feature chunks
    NPAIR = K // 2     # 4 k-pairs per batch
    PF = 3             # batches of prefetch issued ahead
    # process k in pair-arrival order: sync delivers kp0 then kp1,
    # scalar kp2 then kp3, roughly interleaved in time
    KORDER = [0, 1, 4, 5, 2, 3, 6, 7]
    PIPE = 2           # transpose groups issued ahead of their matmuls

    with tile.TileContext(nc) as tc:
        with (
            tc.tile_pool(name="singles", bufs=1) as singles,
            tc.tile_pool(name="p_x", bufs=4) as p_x,
            tc.tile_pool(name="p_xt", bufs=2) as p_xt,
            tc.tile_pool(name="p_h", bufs=4) as p_h,
            tc.tile_pool(name="p_adj", bufs=16) as p_adj,
            tc.tile_pool(name="p_adjt", bufs=8) as p_adjt,
            tc.tile_pool(name="p_out", bufs=4) as p_out,
            tc.tile_pool(name="ps_ta", bufs=4, space=bass.MemorySpace.PSUM) as ps_ta,
            tc.tile_pool(name="ps_h", bufs=2, space=bass.MemorySpace.PSUM) as ps_h,
            tc.tile_pool(name="ps_o", bufs=2, space=bass.MemorySpace.PSUM) as ps_o,
        ):
            # --- constants ---
            id_src = singles.tile([P, P], FP32)
            make_identity(nc, id_src[:])
            id_f = singles.tile([P, P], FP32R)    # identity for fp32r transposes
            nc.vector.tensor_copy(id_f[:], id_src[:])
            id_c = singles.tile([P, P], cdt)      # identity for bf16 transposes
            make_identity(nc, id_c[:])
            ones_row = singles.tile([1, P], cdt)
            nc.gpsimd.memset(ones_row[:], 1.0)
            bias_c = singles.tile([1, OUT_DIM], cdt)
            nc.gpsimd.dma_start(out=bias_c[:], in_=b_ap[None, :])
            w_sb = [singles.tile([P, OUT_DIM], cdt, name=f"w{ic}")
                    for ic in range(IC4)]
            # W before any x: the batch-0 linear blocks the in-order PE
            # program, so W must be resident early
            for ic in range(IC4):
                nc.gpsimd.dma_start(
                    out=w_sb[ic][:], in_=w_ap[ic * P:(ic + 1) * P, :])

            # DRAM views
            # x: [BPC, 128, 2, 512]; partition p <- nodes p, 128+p
            x_v = x_ap.rearrange("b (c p) i -> b p c i", p=P)
            # adj: [BPC, 128, K, 512]; partition p <- rows 2p, 2p+1 of each
            # k slice (contiguous 2KB in HBM)
            adj_v = adj_ap.rearrange("b k (p two) m -> b p k (two m)", two=2)
            # out: [BPC, 2, 128, OUT]; parity-two tile row p <- node 2p+two
            out_v = out_ap.rearrange("b (p two) o -> b two p o", two=2)

            pref = {}

            def prefetch(b):
                a_sbs = []
                for kp in range(NPAIR):
                    eng = nc.sync if kp < NPAIR // 2 else nc.scalar
                    t = p_adj.tile([P, 2 * 2 * N], FP32R, tag="adj",
                                   name=f"a{b}_{kp}")
                    eng.dma_start(out=t[:], in_=adj_v[b, :, 2 * kp:2 * kp + 2])
                    a_sbs.append(t)
                x_sb = p_x.tile([P, NC2 * IN_DIM], cdt, tag="x", name=f"x{b}")
                nc.gpsimd.dma_start(out=x_sb[:], in_=x_v[b])
                pref[b] = (a_sbs, x_sb)

            for b in range(PF):
                prefetch(b)

            for b in range(BPC):
                if b + PF < BPC:
                    prefetch(b + PF)
                a_sbs, x_sb = pref.pop(b)

                # --- transpose x -> xT, both node-chunks staged in ONE
                # psum bank, single DVE drain ---
                xt = p_xt.tile([P, NC2 * IC4 * P], cdt, tag="xT", name=f"xT{b}")
                ptx = ps_ta.tile([P, NC2 * IC4 * P], cdt, tag="pstf",
                                 name=f"ptx{b}")
                for nch in range(NC2):
                    for ic in range(IC4):
                        nc.tensor.transpose(
                            ptx[:, (nch * IC4 + ic) * P:(nch * IC4 + ic + 1) * P],
                            x_sb[:, nch * IN_DIM + ic * P:
                                 nch * IN_DIM + (ic + 1) * P],
                            id_c[:])
                nc.vector.tensor_copy(xt[:], ptx[:])

                def xT_sl(ic, nch):
                    return xt[:, (nch * IC4 + ic) * P:(nch * IC4 + ic + 1) * P]

                # --- linear + bias + relu -> h bf16 [128(n), 512(o)] x2 ---
                h_sb = []
                for nch in range(NC2):
                    ph = ps_h.tile([P, OUT_DIM], FP32, tag="psh", name=f"ph{b}_{nch}")
                    nc.tensor.matmul(ph[:], ones_row[:], bias_c[:],
                                     start=True, stop=False)
                    for ic in range(IC4):
                        nc.tensor.matmul(
                            ph[:], xT_sl(ic, nch), w_sb[ic][:],
                            start=False, stop=(ic == IC4 - 1))
                    ht = p_h.tile([P, OUT_DIM], cdt, tag="h", name=f"h{b}_{nch}")
                    nc.scalar.activation(ht[:], ph[:],
                                         mybir.ActivationFunctionType.Relu)
                    h_sb.append(ht)

                # --- aggregation: transpose groups pipelined PIPE ahead of
                # their matmuls ---
                po = [ps_o.tile([P, OUT_DIM], FP32, tag="pso", name=f"po{b}_{i}")
                      for i in range(2)]
                aTs = {}

                def t_group(gi):
                    k = KORDER[gi]
                    kp, kl = divmod(k, 2)
                    a_sb = a_sbs[kp]
                    aT = p_adjt.tile([P, 4 * P], cdt, tag="adjT",
                                     name=f"aT{b}_{k}")
                    pt = ps_ta.tile([P, 4 * P], FP32R, tag="pstf",
                                    name=f"pta{b}_{k}")
                    for two in range(2):
                        for j in range(2):
                            blk = (two * 2 + j) * P
                            src = kl * 2 * N + two * N + j * P
                            nc.tensor.transpose(
                                pt[:, blk:blk + P], a_sb[:, src:src + P],
                                id_f[:])
                    if gi in (1, 4):
                        nc.scalar.copy(aT[:], pt[:])
                    else:
                        nc.vector.tensor_copy(aT[:], pt[:])
                    aTs[k] = aT

                def m_group(gi):
                    k = KORDER[gi]
                    aT = aTs.pop(k)
                    for two in range(2):
                        for j in range(2):
                            blk = (two * 2 + j) * P
                            nc.tensor.matmul(
                                po[two][:, k * DK:(k + 1) * DK],
                                aT[:, blk:blk + P],
                                h_sb[j][:, k * DK:(k + 1) * DK],
                                start=(j == 0), stop=(j == 1))

                for gi in range(K + PIPE):
                    if gi < K:
                        t_group(gi)
                    if gi >= PIPE:
                        m_group(gi - PIPE)

                # --- drain accumulators (ScalarE cast bf16) + SWDGE store ---
                for two in range(2):
                    ot = p_out.tile([P, OUT_DIM], cdt, tag="o", name=f"o{b}_{two}")
                    nc.scalar.copy(ot[:], po[two][:])
                    nc.gpsimd.dma_start(out=out_v[b, two], in_=ot[:])

    nc.compile()
    return nc


def _get_nc():
    if "nc" not in _compiled:
        _compiled["nc"] = _build()
    return _compiled["nc"]


def _run(inputs, trace=False, trace_cores=None):
    nc = _get_nc()
    node_feats = np.ascontiguousarray(inputs["node_feats"], dtype=np.float32)
    adj = np.ascontiguousarray(inputs["adj"], dtype=np.float32)
    weight = np.ascontiguousarray(inputs["weight"], dtype=np.float32)
    bias = np.ascontiguousarray(inputs["bias"], dtype=np.float32)
    in_maps = []
    for c in range(N_CORES):
        sl = slice(c * BPC, (c + 1) * BPC)
        in_maps.append({
            "node_feats": node_feats[sl],
            "adj": adj[sl],
            "weight": weight,
            "bias": bias,
        })
    res = run_bass_kernel_spmd(
        nc, in_maps, core_ids=list(range(N_CORES)),
        trace=trace, trace_cores=trace_cores)
    out = np.concatenate(
        [np.asarray(res.results[c]["out"]).astype(np.float32)
         for c in range(N_CORES)], axis=0)
    return out.reshape(B, N, OUT_DIM), res


def kernel(**inputs) -> np.ndarray:
    return _run(inputs, trace=False)[0]
